# revision 22
# baseline (speedup 1.0000x reference)
"""Trainium2 Bass kernel for nn_CADenseAdd (context-adaptive low-rank dense + ReLU).

Reference math (per batch row b):
    s_b   = S + context_b @ W                  # [RANK]
    out_b = relu((x_b @ U) * s_b @ V.T + bias) # [UNITS]

Sharding: data-parallel over batch B=2048 across 8 cores (256 rows/core);
U/S/V/W replicated.  All matmuls are done "transposed" so the contraction
dim always lands on SBUF partitions with zero on-device transposes:

    sT  = W^T @ ctxT + S          [RANK,  BS]  (S added on sT eviction, ACT bias)
    xuT = U^T @ xT                [RANK,  BS]
    tT  = xuT * sT  (cast fp16)   [RANK,  BS]
    outT[um] = Vt[um] @ tT        [UNITS, BS]  (ReLU on eviction)

Pipeline (measured ~55 us/core on HW): one HWDGE load stream on the sync
ring in strict need-order (ctx+W merged per-kc -> S (padded to 512 B/part
to dodge the SDMA read-modify-write path) -> x+U merged per-kn -> Vt),
head chunks small so each phase starts as early as possible, tail chunks
small so little compute trails the last DMA-completion semaphore.  ~3.4us
of dummy matmuls (memset source) un-throttle HAM during the DMA head so
all real matmuls run at 2.4 GHz.  mm1 runs first and doubles as a
backlog-builder for the x/U stream; mm1+mm2 use 4+4 PSUM banks, released
before mm3 opens an 8-deep eviction rotation.  All evictions are on DVE
(single-writer staging tiles); one store per um-group on the scalar ring,
preceded by a tiny ACT observer copy whose explicit DVE wait absorbs the
data tick, so the store itself needs only its DMA-lane wait.  The last
group is 2 um-tiles so only a 128KB store trails the final matmul.

Known walls (from NTFF traces): ~7us fixed engine preamble before any
user instruction; DMA chunk-completion semaphores lag the data by
~2-5us (slowest-of-16-engines skew + completion receipt), which gates
mm2's tail and thus mm3's start; ~2.6us store-receipt + drain tail.
"""

import re

import numpy as np

import bass_rust
import concourse.bass as bass
import concourse.tile as tile
from concourse import mybir
from concourse.bass_utils import run_bass_kernel_spmd
from concourse.vector_clock import ScopedClock


def _split_drain_and_barrier(self, tick_clock, wait_clock):
    """Replacement for TileContext._drain_and_barrier.

    The walrus build in this toolchain cannot encode more than one sync
    wait per instruction ("Too many sync wait commands"), and Tile's final
    drain carries one wait per active proc (~12 here).  Emit those waits as
    a chain of single-wait SP nops instead, then a bare drain: the SP queue
    executes in order, so the drain still happens after every proc's final
    tick.
    """
    ticks = [int(x) for x in re.findall(r"\d+", repr(tick_clock.global_clock))]
    for proc, tick in enumerate(ticks):
        if tick > 0:
            nop_inst = self.nc.sync.nop(nofuse=True)
            sub = bass_rust.VectorClock()
            sub.require_at_least(proc, tick)
            wait_clock.add_sem_waits(nop_inst.ins, ScopedClock({None: sub}))
    self.nc.sync.drain()
    self.nc.all_engine_barrier()
    popped = self.nc._tile_sem_poison_stack.pop()
    assert popped is self._sem_poison
    self.nc.clear_and_free_semaphores(list(self.sems.allocated().values()))
    self.nc.all_engine_barrier()


tile.TileContext._drain_and_barrier = _split_drain_and_barrier

# Problem shape (hardcoded per contract)
M = 8  # cores
B, N, C = 2048, 4096, 1024
UNITS, RANK = 4096, 512
BS = B // M  # 256 rows per core
P = 128
KN = N // P      # 32 contraction tiles for x @ U
KC = C // P      # 8 contraction tiles for ctx @ W
RM = RANK // P   # 4 tiles of RANK
UM = UNITS // P  # 32 tiles of UNITS
FW = BS + RANK   # merged ctx|W and x|U free width (768)

F16 = mybir.dt.float16
F32 = mybir.dt.float32

# load-stream chunking (need-order). Head chunks small so mm1/mm2/mm3
# start early; later chunks big for DMA efficiency.
CW_CHUNKS = [(0, 2), (2, 4), (4, 8)]
XU_CHUNKS = [(0, 2), (2, 4), (4, 8), (8, 16), (16, 24), (24, 30), (30, 32)]
V_CHUNKS = [(0, 2), (2, 4), (4, 8), (8, 16), (16, 24), (24, 32)]
SPAD = 128  # S padded to 512 B/partition so its DMA avoids the SDMA RMW path
OUT_GROUPS = [8, 8, 8, 6, 2]

N_WARM_MM = 16  # ~3.4us of cold-rate matmuls: spans the HAM SHORT window


def build_program(zero_bias: bool = True) -> bass.Bass:
    """Build the per-core SPMD program.

    Wait-encoding constraint: this walrus build cannot encode >1 sem-wait
    on DVE/ACT tensor instructions, while matmuls can encode 2.  Every
    DVE/ACT instruction below keeps <=1 wait: each engine "pre-touches"
    its DMA-sourced operands once, phase-boundary fences absorb cross-
    engine ticks, and output staging tiles are never reused.
    """
    nc = bass.Bass("TRN2", debug=False, enable_asserts=False, enable_partition_id=False, dynamic_dma_scratch_size=4096)

    cwT_d = nc.dram_tensor("cwT", [P, KC, FW], F16, kind="ExternalInput").ap()
    xuT_d = nc.dram_tensor("xuT", [P, KN, FW], F16, kind="ExternalInput").ap()
    V3_d = nc.dram_tensor("V3", [P, UM, RM, P], F16, kind="ExternalInput").ap()
    S_d = nc.dram_tensor("Spk", [P, SPAD], F32, kind="ExternalInput").ap()
    if not zero_bias:
        bias_d = nc.dram_tensor("bias", [P, UM], F32, kind="ExternalInput").ap()
    outT_d = nc.dram_tensor("outT", [P, UM, BS], F16, kind="ExternalOutput").ap()

    # Pre-Tile head load: cw streams during the ~7us engine preamble on a
    # raw semaphore, so mm1's operands are resident before the Tile stream
    # even starts and xu owns the in-Tile ring from byte 0.
    head_sem = nc.alloc_semaphore("head_dma")
    cw_h = nc.alloc_sbuf_tensor("cw_sb", [P, KC, FW], F16)
    cw_sb = cw_h.ap()
    nc.sync.dma_start(cw_sb[:], cwT_d[:]).then_inc(head_sem, 16)
    # PE observes the completion in the parent block, before any Tile
    # instruction, so mm1's matmuls need no DMA waits at all.
    nc.tensor.wait_ge(head_sem, 16)

    with tile.TileContext(nc) as tc:
        with (
            tc.tile_pool(name="consts", bufs=1) as cpool,
            tc.tile_pool(name="xup", bufs=1) as xupool,
            tc.tile_pool(name="vp", bufs=1) as vpool,
            tc.tile_pool(name="actp", bufs=1) as actpool,
            tc.tile_pool(name="op", bufs=1) as opool,
        ):
            ps_s_pool = tc.alloc_tile_pool(name="pss", bufs=4, space="PSUM")
            ps_xu_pool = tc.alloc_tile_pool(name="psxu", bufs=4, space="PSUM")

            # ---- input loads, all on the sync ring in need-order.  S is
            # tiny and only needed by the sT adds (mid-mm2), so it rides
            # after the xu chunks to keep the stream head fat.
            def _load(dst, srcap):
                nc.sync.dma_start(dst, srcap)

            S_sb = cpool.tile([P, SPAD], F32, name="S_sb")
            _load(S_sb[:], S_d[:])
            xu_sb = xupool.tile([P, KN, FW], F16, name="xu_sb")
            for lo, hi in XU_CHUNKS:
                _load(xu_sb[:, lo:hi, :], xuT_d[:, lo:hi, :])
            if not zero_bias:
                b_sb = cpool.tile([P, UM], F32, name="b_sb")
                _load(b_sb[:], bias_d[:])
            v_sb = vpool.tile([P, UM, RM, P], F16, name="v_sb")
            for lo, hi in V_CHUNKS:
                _load(v_sb[:, lo:hi, :, :], V3_d[:, lo:hi, :, :])

            # ---- engine warm-up during the DMA fill: ~3.4us of dummy
            # matmuls un-throttles HAM before mm1, so all real matmuls run
            # at 2.4 GHz.
            warm_src = cpool.tile([P, BS + P], F16, name="warm_src")
            nc.vector.memset(warm_src[:], 0.0)
            ps_warm = ps_s_pool.tile([P, BS], F32, name="ps_warm", tag="s")
            for _ in range(N_WARM_MM):
                nc.tensor.matmul(
                    ps_warm[:], lhsT=warm_src[:, BS:], rhs=warm_src[:, :BS],
                    start=True, stop=True,
                )

            # ---- mm1: sT = W^T @ ctxT, kc-outer into 4 banks; also builds
            # xu-stream backlog so mm2 runs stall-free at full PE rate.
            ps_s = [
                ps_s_pool.tile([P, BS], F32, name=f"ps_s{rm}", tag="s")
                for rm in range(RM)
            ]
            for kc in range(KC):
                for rm in range(RM):
                    nc.tensor.matmul(
                        ps_s[rm][:],
                        lhsT=cw_sb[:, kc, BS + rm * P : BS + (rm + 1) * P],
                        rhs=cw_sb[:, kc, :BS],
                        start=(kc == 0),
                        stop=(kc == KC - 1),
                    )
            # ACT pre-touch: loads the Relu table during mm1 so the first
            # real eviction doesn't stall on the table DMA.
            act_scr = cpool.tile([P, 1], F16, name="act_scr")
            nc.scalar.activation(
                act_scr[:], warm_src[:, :1],
                mybir.ActivationFunctionType.Relu, bias=0.0,
            )
            if not zero_bias:
                act_scr2 = cpool.tile([P, UM], F32, name="act_scr2")
                nc.scalar.copy(act_scr2[:], b_sb[:])
                dve_scr2 = cpool.tile([P, UM], F32, name="dve_scr2")
                nc.vector.tensor_copy(dve_scr2[:], b_sb[:])

            # ---- mm2: xuT = U^T @ xT, kn-outer into 4 banks ----
            ps_xu = [
                ps_xu_pool.tile([P, BS], F32, name=f"ps_xu{rm}", tag="xu")
                for rm in range(RM)
            ]
            for kn in range(KN):
                for rm in range(RM):
                    nc.tensor.matmul(
                        ps_xu[rm][:],
                        lhsT=xu_sb[:, kn, BS + rm * P : BS + (rm + 1) * P],
                        rhs=xu_sb[:, kn, :BS],
                        start=(kn == 0),
                        stop=(kn == KN - 1),
                    )
            # sT[rm] = ps_s[rm] + S on DVE: issued after mm2 but executes
            # during it (DVE is idle; ps_s banks are untouched by mm2).
            # DVE pre-touch absorbs S's DMA tick so the adds keep <=1 wait.
            dve_scr = cpool.tile([P, RM], F32, name="dve_scr")
            nc.vector.tensor_copy(dve_scr[:, :1], S_sb[:, :1])
            sT = [actpool.tile([P, BS], F32, name=f"sT{rm}") for rm in range(RM)]
            for rm in range(RM):
                nc.vector.tensor_tensor(
                    sT[rm][:], ps_s[rm][:],
                    S_sb[:, rm : rm + 1].to_broadcast((P, BS)),
                    mybir.AluOpType.add,
                )
            # DVE fence: observe sT3's completion tick so the tT multiplies
            # need only their PE wait.
            nc.vector.tensor_copy(dve_scr[:, 1:2], sT[RM - 1][:, :1])
            tT = [actpool.tile([P, BS], F16, name=f"tT{rm}") for rm in range(RM)]
            for rm in range(RM):
                nc.vector.tensor_mul(tT[rm][:], ps_xu[rm][:], sT[rm][:])

            ps_xu_pool.release()
            ps_s_pool.release()
            ps_o_pool = tc.alloc_tile_pool(name="pso", bufs=8, space="PSUM")

            # Phase-boundary fences: every reader of the released PSUM
            # banks is DVE, so a single PE ldweights observing the last DVE
            # tick absorbs them all, and one dummy matmul absorbs the
            # released-bank WAW tick — every mm3 matmul keeps <=1 wait.
            nc.tensor.ldweights(tT[RM - 1][:, :P])
            ps_fence = ps_o_pool.tile([P, BS], F32, name="ps_fence", tag="pso")
            nc.tensor.matmul(
                ps_fence[:], lhsT=tT[RM - 1][:, :P], rhs=tT[RM - 1][:],
                start=True, stop=True,
            )
            # DVE/ACT fences: absorb the released-bank accessor ticks so
            # each eviction needs only its PE wait.
            nc.vector.tensor_copy(dve_scr[:, 2:3], tT[RM - 1][:, :1])
            act_fence_scr = cpool.tile([P, 1], F16, name="act_fence_scr")
            nc.scalar.copy(act_fence_scr[:], tT[RM - 1][:, :1])

            # ---- mm3: outT[um] = relu(Vt[um] @ tT + bias[um]) ----
            # All evictions on DVE (so every staging tile has a single
            # non-ACT writer); one store per group on the scalar ring,
            # preceded by a tiny ACT observer copy of the last DVE-written
            # slice: the explicit DVE wait on the observer absorbs the data
            # tick, so the store itself needs only its DMA-lane wait.
            um0 = 0
            for g, gs in enumerate(OUT_GROUPS):
                og = opool.tile([P, gs, BS], F16, name=f"og{g}")
                obs = cpool.tile([P, 1], F16, name=f"obs{g}")
                for j in range(gs):
                    um = um0 + j
                    ps_o = ps_o_pool.tile([P, BS], F32, name="ps_o", tag="pso")
                    vt = v_sb[:, um, :, :]  # [P, RM, P]
                    for kr in range(RM):
                        nc.tensor.matmul(
                            ps_o[:],
                            lhsT=vt[:, kr, :],
                            rhs=tT[kr][:],
                            start=(kr == 0),
                            stop=(kr == RM - 1),
                        )
                    if zero_bias:
                        nc.vector.tensor_scalar_max(og[:, j, :], ps_o[:], 0.0)
                    else:
                        nc.vector.tensor_tensor(
                            og[:, j, :], ps_o[:],
                            b_sb[:, um : um + 1].to_broadcast((P, BS)),
                            mybir.AluOpType.add,
                        )
                        nc.vector.tensor_scalar_max(
                            og[:, j, :], og[:, j, :], 0.0
                        )
                nc.scalar.copy(obs[:], og[:, gs - 1, :1])
                nc.scalar.dma_start(outT_d[:, um0 : um0 + gs, :], og[:])
                um0 += gs
            assert um0 == UM

            ps_o_pool.release()

        pass

    nc.clear_and_free_semaphores([head_sem])
    return nc


def _pack_inputs(inputs, context, U, S, V, W, bias):
    """Shard + pack the full fp32 inputs into per-core [128,...] fp16 layouts.

    ctx|W and x|U are merged along the free dim so each need-order chunk is
    a single DMA; S rides separately as a [128, RM] fp32 bias tile.
    """
    zero_bias = not bias.any()
    x16 = inputs.astype(np.float16)
    c16 = context.astype(np.float16)
    U_pk = U.astype(np.float16).reshape(KN, P, RANK).transpose(1, 0, 2)
    W_pk = W.astype(np.float16).reshape(KC, P, RANK).transpose(1, 0, 2)
    S_pk = np.zeros((P, SPAD), dtype=np.float32)
    S_pk[:, :RM] = S.astype(np.float32).reshape(RM, P).T
    # V3[p, um, kr, c] = V[um*128 + c, kr*128 + p]
    V3_pk = np.ascontiguousarray(
        V.astype(np.float16).reshape(UM, P, RM, P).transpose(3, 0, 2, 1)
    )
    b_pk = np.ascontiguousarray(bias.astype(np.float32).reshape(UM, P).T)

    in_maps = []
    for c in range(M):
        xs = x16[c * BS : (c + 1) * BS]  # [BS, N]
        cs = c16[c * BS : (c + 1) * BS]  # [BS, C]
        xuT = np.empty((P, KN, FW), dtype=np.float16)
        xuT[:, :, :BS] = xs.T.reshape(KN, P, BS).transpose(1, 0, 2)
        xuT[:, :, BS:] = U_pk
        cwT = np.empty((P, KC, FW), dtype=np.float16)
        cwT[:, :, :BS] = cs.T.reshape(KC, P, BS).transpose(1, 0, 2)
        cwT[:, :, BS:] = W_pk
        im = {"cwT": cwT, "xuT": xuT, "V3": V3_pk, "Spk": S_pk}
        if not zero_bias:
            im["bias"] = b_pk
        in_maps.append(im)
    return in_maps


_PROGRAM_CACHE = {}


def _get_program(zero_bias: bool) -> bass.Bass:
    if zero_bias not in _PROGRAM_CACHE:
        _PROGRAM_CACHE[zero_bias] = build_program(zero_bias=zero_bias)
    return _PROGRAM_CACHE[zero_bias]


def _unpack_outputs(results) -> np.ndarray:
    shards = []
    for r in results:
        outT = r["outT"]  # [P, UM, BS] fp16
        shards.append(outT.transpose(1, 0, 2).reshape(UNITS, BS).T)
    return np.concatenate(shards, axis=0).astype(np.float32)


def kernel(inputs, context, U, S, V, W, bias, _trace=False):
    bias = np.asarray(bias)
    in_maps = _pack_inputs(
        np.asarray(inputs), np.asarray(context), np.asarray(U),
        np.asarray(S), np.asarray(V), np.asarray(W), bias,
    )
    nc = _get_program(zero_bias=not bias.any())
    res = run_bass_kernel_spmd(nc, in_maps, core_ids=list(range(M)), trace=_trace)
    out = _unpack_outputs(res.results)
    if _trace:
        return out, res
    return out


# revision 24
# speedup vs baseline: 1.0284x; 1.0284x over previous
"""Trainium2 Bass kernel for nn_CADenseAdd (context-adaptive low-rank dense + ReLU).

Reference math (per batch row b):
    s_b   = S + context_b @ W                  # [RANK]
    out_b = relu((x_b @ U) * s_b @ V.T + bias) # [UNITS]

Sharding: data-parallel over batch B=2048 across 8 cores (256 rows/core);
U/S/V/W replicated.  All matmuls are done "transposed" so the contraction
dim always lands on SBUF partitions with zero on-device transposes:

    sT  = W^T @ ctxT + S          [RANK,  BS]  (S added on sT eviction, ACT bias)
    xuT = U^T @ xT                [RANK,  BS]
    tT  = xuT * sT  (cast fp16)   [RANK,  BS]
    outT[um] = Vt[um] @ tT        [UNITS, BS]  (ReLU on eviction)

Pipeline (measured ~55 us/core on HW): one HWDGE load stream on the sync
ring in strict need-order (ctx+W merged per-kc -> S (padded to 512 B/part
to dodge the SDMA read-modify-write path) -> x+U merged per-kn -> Vt),
head chunks small so each phase starts as early as possible, tail chunks
small so little compute trails the last DMA-completion semaphore.  ~3.4us
of dummy matmuls (memset source) un-throttle HAM during the DMA head so
all real matmuls run at 2.4 GHz.  mm1 runs first and doubles as a
backlog-builder for the x/U stream; mm1+mm2 use 4+4 PSUM banks, released
before mm3 opens an 8-deep eviction rotation.  All evictions are on DVE
(single-writer staging tiles); one store per um-group on the scalar ring,
preceded by a tiny ACT observer copy whose explicit DVE wait absorbs the
data tick, so the store itself needs only its DMA-lane wait.  The last
group is 2 um-tiles so only a 128KB store trails the final matmul.

Known walls (from NTFF traces): ~7us fixed engine preamble before any
user instruction; DMA chunk-completion semaphores lag the data by
~2-5us (slowest-of-16-engines skew + completion receipt), which gates
mm2's tail and thus mm3's start; ~2.6us store-receipt + drain tail.
"""

import re

import numpy as np

import bass_rust
import concourse.bass as bass
import concourse.tile as tile
from concourse import mybir
from concourse.bass_utils import run_bass_kernel_spmd
from concourse.vector_clock import ScopedClock


def _split_drain_and_barrier(self, tick_clock, wait_clock):
    """Replacement for TileContext._drain_and_barrier.

    The walrus build in this toolchain cannot encode more than one sync
    wait per instruction ("Too many sync wait commands"), and Tile's final
    drain carries one wait per active proc (~12 here).  Emit those waits as
    a chain of single-wait SP nops instead, then a bare drain: the SP queue
    executes in order, so the drain still happens after every proc's final
    tick.
    """
    ticks = [int(x) for x in re.findall(r"\d+", repr(tick_clock.global_clock))]
    for proc, tick in enumerate(ticks):
        if tick > 0:
            nop_inst = self.nc.sync.nop(nofuse=True)
            sub = bass_rust.VectorClock()
            sub.require_at_least(proc, tick)
            wait_clock.add_sem_waits(nop_inst.ins, ScopedClock({None: sub}))
    self.nc.sync.drain()
    self.nc.all_engine_barrier()
    popped = self.nc._tile_sem_poison_stack.pop()
    assert popped is self._sem_poison
    self.nc.clear_and_free_semaphores(list(self.sems.allocated().values()))


tile.TileContext._drain_and_barrier = _split_drain_and_barrier

# Problem shape (hardcoded per contract)
M = 8  # cores
B, N, C = 2048, 4096, 1024
UNITS, RANK = 4096, 512
BS = B // M  # 256 rows per core
P = 128
KN = N // P      # 32 contraction tiles for x @ U
KC = C // P      # 8 contraction tiles for ctx @ W
RM = RANK // P   # 4 tiles of RANK
UM = UNITS // P  # 32 tiles of UNITS
FW = BS + RANK   # merged ctx|W and x|U free width (768)

F16 = mybir.dt.float16
F32 = mybir.dt.float32

# load-stream chunking (need-order). Head chunks small so mm1/mm2/mm3
# start early; later chunks big for DMA efficiency.
CW_CHUNKS = [(0, 2), (2, 4), (4, 8)]
XU_CHUNKS = [(0, 2), (2, 4), (4, 8), (8, 16), (16, 24), (24, 30), (30, 32)]
V_CHUNKS = [(0, 2), (2, 4), (4, 8), (8, 16), (16, 24), (24, 32)]
SPAD = 128  # S padded to 512 B/partition so its DMA avoids the SDMA RMW path
OUT_GROUPS = [8, 8, 8, 6, 2]

N_WARM_MM = 16  # ~3.4us of cold-rate matmuls: spans the HAM SHORT window


def build_program(zero_bias: bool = True) -> bass.Bass:
    """Build the per-core SPMD program.

    Wait-encoding constraint: this walrus build cannot encode >1 sem-wait
    on DVE/ACT tensor instructions, while matmuls can encode 2.  Every
    DVE/ACT instruction below keeps <=1 wait: each engine "pre-touches"
    its DMA-sourced operands once, phase-boundary fences absorb cross-
    engine ticks, and output staging tiles are never reused.
    """
    nc = bass.Bass("TRN2", debug=False, enable_asserts=False, enable_partition_id=False, dynamic_dma_scratch_size=4096)

    cwT_d = nc.dram_tensor("cwT", [P, KC, FW], F16, kind="ExternalInput").ap()
    xuT_d = nc.dram_tensor("xuT", [P, KN, FW], F16, kind="ExternalInput").ap()
    V3_d = nc.dram_tensor("V3", [P, UM, RM, P], F16, kind="ExternalInput").ap()
    S_d = nc.dram_tensor("Spk", [P, SPAD], F32, kind="ExternalInput").ap()
    if not zero_bias:
        bias_d = nc.dram_tensor("bias", [P, UM], F32, kind="ExternalInput").ap()
    outT_d = nc.dram_tensor("outT", [P, UM, BS], F16, kind="ExternalOutput").ap()

    with tile.TileContext(nc) as tc:
        with (
            tc.tile_pool(name="consts", bufs=1) as cpool,
            tc.tile_pool(name="cwp", bufs=1) as cwpool,
            tc.tile_pool(name="xup", bufs=1) as xupool,
            tc.tile_pool(name="vp", bufs=1) as vpool,
            tc.tile_pool(name="actp", bufs=1) as actpool,
            tc.tile_pool(name="op", bufs=1) as opool,
        ):
            ps_s_pool = tc.alloc_tile_pool(name="pss", bufs=4, space="PSUM")
            ps_xu_pool = tc.alloc_tile_pool(name="psxu", bufs=4, space="PSUM")

            # ---- input loads, all on the sync ring in need-order.  S is
            # tiny and only needed by the sT adds (mid-mm2), so it rides
            # after the xu chunks to keep the stream head fat.
            def _load(dst, srcap):
                nc.sync.dma_start(dst, srcap)

            cw_sb = cwpool.tile([P, KC, FW], F16, name="cw_sb")
            for lo, hi in CW_CHUNKS:
                _load(cw_sb[:, lo:hi, :], cwT_d[:, lo:hi, :])
            S_sb = cpool.tile([P, SPAD], F32, name="S_sb")
            _load(S_sb[:], S_d[:])
            xu_sb = xupool.tile([P, KN, FW], F16, name="xu_sb")
            for lo, hi in XU_CHUNKS:
                _load(xu_sb[:, lo:hi, :], xuT_d[:, lo:hi, :])
            if not zero_bias:
                b_sb = cpool.tile([P, UM], F32, name="b_sb")
                _load(b_sb[:], bias_d[:])
            v_sb = vpool.tile([P, UM, RM, P], F16, name="v_sb")
            for lo, hi in V_CHUNKS:
                _load(v_sb[:, lo:hi, :, :], V3_d[:, lo:hi, :, :])

            # ---- engine warm-up during the DMA fill: ~3.4us of dummy
            # matmuls un-throttles HAM before mm1, so all real matmuls run
            # at 2.4 GHz.
            warm_src = cpool.tile([P, BS + P], F16, name="warm_src")
            nc.vector.memset(warm_src[:], 0.0)
            ps_warm = ps_s_pool.tile([P, BS], F32, name="ps_warm", tag="s")
            for _ in range(N_WARM_MM):
                nc.tensor.matmul(
                    ps_warm[:], lhsT=warm_src[:, BS:], rhs=warm_src[:, :BS],
                    start=True, stop=True,
                )

            # ---- mm1: sT = W^T @ ctxT, kc-outer into 4 banks.  Runs as
            # soon as the first cw chunk lands; also builds xu-stream
            # backlog so mm2 runs stall-free at full PE rate.
            ps_s = [
                ps_s_pool.tile([P, BS], F32, name=f"ps_s{rm}", tag="s")
                for rm in range(RM)
            ]
            for kc in range(KC):
                for rm in range(RM):
                    nc.tensor.matmul(
                        ps_s[rm][:],
                        lhsT=cw_sb[:, kc, BS + rm * P : BS + (rm + 1) * P],
                        rhs=cw_sb[:, kc, :BS],
                        start=(kc == 0),
                        stop=(kc == KC - 1),
                    )
            # ACT pre-touch: loads the Relu table during mm1 so the first
            # real eviction doesn't stall on the table DMA.
            act_scr = cpool.tile([P, 1], F16, name="act_scr")
            nc.scalar.activation(
                act_scr[:], warm_src[:, :1],
                mybir.ActivationFunctionType.Relu, bias=0.0,
            )
            if not zero_bias:
                act_scr2 = cpool.tile([P, UM], F32, name="act_scr2")
                nc.scalar.copy(act_scr2[:], b_sb[:])
                dve_scr2 = cpool.tile([P, UM], F32, name="dve_scr2")
                nc.vector.tensor_copy(dve_scr2[:], b_sb[:])

            # ---- mm2: xuT = U^T @ xT, kn-outer into 4 banks ----
            ps_xu = [
                ps_xu_pool.tile([P, BS], F32, name=f"ps_xu{rm}", tag="xu")
                for rm in range(RM)
            ]
            for kn in range(KN):
                for rm in range(RM):
                    nc.tensor.matmul(
                        ps_xu[rm][:],
                        lhsT=xu_sb[:, kn, BS + rm * P : BS + (rm + 1) * P],
                        rhs=xu_sb[:, kn, :BS],
                        start=(kn == 0),
                        stop=(kn == KN - 1),
                    )
            # sT[rm] = ps_s[rm] + S on DVE: issued after mm2 but executes
            # during it (DVE is idle; ps_s banks are untouched by mm2).
            # DVE pre-touch absorbs S's DMA tick so the adds keep <=1 wait.
            dve_scr = cpool.tile([P, RM], F32, name="dve_scr")
            nc.vector.tensor_copy(dve_scr[:, :1], S_sb[:, :1])
            sT = [actpool.tile([P, BS], F32, name=f"sT{rm}") for rm in range(RM)]
            for rm in range(RM):
                nc.vector.tensor_tensor(
                    sT[rm][:], ps_s[rm][:],
                    S_sb[:, rm : rm + 1].to_broadcast((P, BS)),
                    mybir.AluOpType.add,
                )
            # DVE fence: observe sT3's completion tick so the tT multiplies
            # need only their PE wait.
            nc.vector.tensor_copy(dve_scr[:, 1:2], sT[RM - 1][:, :1])
            tT = [actpool.tile([P, BS], F16, name=f"tT{rm}") for rm in range(RM)]
            for rm in range(RM):
                nc.vector.tensor_mul(tT[rm][:], ps_xu[rm][:], sT[rm][:])

            ps_xu_pool.release()
            ps_s_pool.release()
            ps_o_pool = tc.alloc_tile_pool(name="pso", bufs=8, space="PSUM")

            # Phase-boundary fences: every reader of the released PSUM
            # banks is DVE, so a single PE ldweights observing the last DVE
            # tick absorbs them all, and one dummy matmul absorbs the
            # released-bank WAW tick — every mm3 matmul keeps <=1 wait.
            nc.tensor.ldweights(tT[RM - 1][:, :P])
            ps_fence = ps_o_pool.tile([P, BS], F32, name="ps_fence", tag="pso")
            nc.tensor.matmul(
                ps_fence[:], lhsT=tT[RM - 1][:, :P], rhs=tT[RM - 1][:],
                start=True, stop=True,
            )
            # DVE/ACT fences: absorb the released-bank accessor ticks so
            # each eviction needs only its PE wait.
            nc.vector.tensor_copy(dve_scr[:, 2:3], tT[RM - 1][:, :1])
            act_fence_scr = cpool.tile([P, 1], F16, name="act_fence_scr")
            nc.scalar.copy(act_fence_scr[:], tT[RM - 1][:, :1])

            # ---- mm3: outT[um] = relu(Vt[um] @ tT + bias[um]) ----
            # All evictions on DVE (so every staging tile has a single
            # non-ACT writer); one store per group on the scalar ring,
            # preceded by a tiny ACT observer copy of the last DVE-written
            # slice: the explicit DVE wait on the observer absorbs the data
            # tick, so the store itself needs only its DMA-lane wait.
            um0 = 0
            for g, gs in enumerate(OUT_GROUPS):
                og_d = opool.tile([P, gs // 2, BS], F16, name=f"ogd{g}")
                og_a = opool.tile([P, gs // 2, BS], F16, name=f"oga{g}")
                obs_d = cpool.tile([P, 1], F16, name=f"obsd{g}")
                obs_a = cpool.tile([P, 1], F16, name=f"obsa{g}")
                for j in range(gs):
                    um = um0 + j
                    ps_o = ps_o_pool.tile([P, BS], F32, name="ps_o", tag="pso")
                    vt = v_sb[:, um, :, :]  # [P, RM, P]
                    for kr in range(RM):
                        nc.tensor.matmul(
                            ps_o[:],
                            lhsT=vt[:, kr, :],
                            rhs=tT[kr][:],
                            start=(kr == 0),
                            stop=(kr == RM - 1),
                        )
                    if zero_bias:
                        if j % 2 == 0:
                            nc.vector.tensor_scalar_max(
                                og_d[:, j // 2, :], ps_o[:], 0.0
                            )
                        else:
                            nc.scalar.activation(
                                og_a[:, j // 2, :], ps_o[:],
                                mybir.ActivationFunctionType.Relu, bias=0.0,
                            )
                    else:
                        if j % 2 == 0:
                            nc.vector.tensor_tensor(
                                og_d[:, j // 2, :], ps_o[:],
                                b_sb[:, um : um + 1].to_broadcast((P, BS)),
                                mybir.AluOpType.add,
                            )
                            nc.vector.tensor_scalar_max(
                                og_d[:, j // 2, :], og_d[:, j // 2, :], 0.0
                            )
                        else:
                            nc.scalar.activation(
                                og_a[:, j // 2, :], ps_o[:],
                                mybir.ActivationFunctionType.Relu,
                                bias=b_sb[:, um : um + 1],
                            )
                # interleaved store-back: even um slices from og_d on the
                # scalar ring (ACT obs absorbs the DVE tick), odd from og_a
                # on the gpsimd SWDGE ring (gpsimd obs absorbs the ACT tick)
                out_g = outT_d[:, um0 : um0 + gs, :].rearrange(
                    "p (o two) b -> p o two b", two=2
                )
                nc.scalar.copy(obs_d[:], og_d[:, gs // 2 - 1, :1])
                nc.scalar.dma_start(out_g[:, :, 0, :], og_d[:])
                nc.gpsimd.tensor_copy(obs_a[:], og_a[:, gs // 2 - 1, :1])
                nc.gpsimd.dma_start(out_g[:, :, 1, :], og_a[:])
                um0 += gs
            assert um0 == UM

            ps_o_pool.release()

    return nc


def _pack_inputs(inputs, context, U, S, V, W, bias):
    """Shard + pack the full fp32 inputs into per-core [128,...] fp16 layouts.

    ctx|W and x|U are merged along the free dim so each need-order chunk is
    a single DMA; S rides separately as a [128, RM] fp32 bias tile.
    """
    zero_bias = not bias.any()
    x16 = inputs.astype(np.float16)
    c16 = context.astype(np.float16)
    U_pk = U.astype(np.float16).reshape(KN, P, RANK).transpose(1, 0, 2)
    W_pk = W.astype(np.float16).reshape(KC, P, RANK).transpose(1, 0, 2)
    S_pk = np.zeros((P, SPAD), dtype=np.float32)
    S_pk[:, :RM] = S.astype(np.float32).reshape(RM, P).T
    # V3[p, um, kr, c] = V[um*128 + c, kr*128 + p]
    V3_pk = np.ascontiguousarray(
        V.astype(np.float16).reshape(UM, P, RM, P).transpose(3, 0, 2, 1)
    )
    b_pk = np.ascontiguousarray(bias.astype(np.float32).reshape(UM, P).T)

    in_maps = []
    for c in range(M):
        xs = x16[c * BS : (c + 1) * BS]  # [BS, N]
        cs = c16[c * BS : (c + 1) * BS]  # [BS, C]
        xuT = np.empty((P, KN, FW), dtype=np.float16)
        xuT[:, :, :BS] = xs.T.reshape(KN, P, BS).transpose(1, 0, 2)
        xuT[:, :, BS:] = U_pk
        cwT = np.empty((P, KC, FW), dtype=np.float16)
        cwT[:, :, :BS] = cs.T.reshape(KC, P, BS).transpose(1, 0, 2)
        cwT[:, :, BS:] = W_pk
        im = {"cwT": cwT, "xuT": xuT, "V3": V3_pk, "Spk": S_pk}
        if not zero_bias:
            im["bias"] = b_pk
        in_maps.append(im)
    return in_maps


_PROGRAM_CACHE = {}


def _get_program(zero_bias: bool) -> bass.Bass:
    if zero_bias not in _PROGRAM_CACHE:
        _PROGRAM_CACHE[zero_bias] = build_program(zero_bias=zero_bias)
    return _PROGRAM_CACHE[zero_bias]


def _unpack_outputs(results) -> np.ndarray:
    shards = []
    for r in results:
        outT = r["outT"]  # [P, UM, BS] fp16
        shards.append(outT.transpose(1, 0, 2).reshape(UNITS, BS).T)
    return np.concatenate(shards, axis=0).astype(np.float32)


def kernel(inputs, context, U, S, V, W, bias, _trace=False):
    bias = np.asarray(bias)
    in_maps = _pack_inputs(
        np.asarray(inputs), np.asarray(context), np.asarray(U),
        np.asarray(S), np.asarray(V), np.asarray(W), bias,
    )
    nc = _get_program(zero_bias=not bias.any())
    res = run_bass_kernel_spmd(nc, in_maps, core_ids=list(range(M)), trace=_trace)
    out = _unpack_outputs(res.results)
    if _trace:
        return out, res
    return out
